# revision 16
# baseline (speedup 1.0000x reference)
"""Trainium2 Bass kernel for nn_CNN_88098369175781.

Model: x[1,1,18,T=262144] -> wavA=x[...,0,:], eeg=x[...,1:17,:], wavB=x[...,17,:]
  wav streams: proj(1->16, pointwise) -> diagonal sinc filter bank (15 taps,
  pad 7) -> conv(16->10, 9 taps) + bias -> relu -> global max-pool.
  eeg stream:  conv(16->10, 9 taps) + bias -> relu -> global max-pool.
  concat -> sigmoid FC(30->30) -> sigmoid FC(30->2).

Device decomposition (validated vs reference in numpy):
  * Each wav stream's three linear stages fuse into ONE 1->10 channel, 23-tap
    conv on the zero-padded raw wav signal (weights precomposed on host).
  * Bias/relu commute past the global max, so the device only computes
    convs + maxima.
  * Convs run on the tensor engine via a polyphase formulation:
      eeg:  time phases r in [0,8), outputs (o, dt in [0,8)) => M=80,
            contraction (c,r) => K=128, 2 accumulating matmuls (u-groups).
      wav:  time phases v in [0,12), outputs (o, dt in [0,12)) => M=120,
            contraction (v,q in [0,3)) => K=36, single matmul per tile.
  * Operands fp16 (PSUM accumulation fp32). 8 cores split the time axis.
  * Host combines per-core maxima and runs the tiny FC head.

Perf structure (v3), from measured hardware rates:
  * DMA: one dma_start per queue (per-queue turnaround between dma_starts is
    ~2us): sync gets wts+eeg[0:2049] merged into one DRAM tensor, scalar gets
    eeg[2048:4097], gpsimd (software DGE) streams wavP.
  * PE warmup: back-to-back matmuls from the first instruction slot -- the
    HAM clock gate needs ~3.4us of SUSTAINED activity to reach 2.4 GHz.
  * PSUM evacuation (measured: reduce=1.11ns/col on any dtype; fp16 SBUF
    tensor_tensor max = 0.33ns/out-col; ACT cast=0.96ns/col):
    ACT casts five 1024-col PSUM tiles to fp16 SBUF; DVE folds each group
    with cheap fp16 pair-max trees, direct-reduces the rest, and one 3D-AP
    reduce folds all tree remnants in a single instruction.
"""

import os
import numpy as np

T = 262144
NOUT = T - 8            # 262136 valid conv output positions
NCORES = 8
KLEN = 15
SIGMA = 0.005

EEG_NCOL = 4096         # eeg matmul columns per core (8 outputs each)
EEG_COLS = EEG_NCOL + 1  # phase row length (g=1 needs one extra column)
WAV_NCOL = 2731         # wav matmul columns per core (12 outputs each)
EEG_TC = 8 * EEG_NCOL   # 32768 eeg outputs per core
WAV_TC = 12 * WAV_NCOL  # 32772 wav outputs per core

_NC_CACHE = {}
LAST_RESULT = None      # BassKernelResults of the most recent device run


# --------------------------------------------------------------------------
# host-side weight precompute
# --------------------------------------------------------------------------

def _sinc_rows(mu):
    """Diagonal rows of the reference's sinc_kernel: [16, 15] float64."""
    k = np.linspace(-1.0, 1.0, KLEN)
    kk = (k[None, :] - np.asarray(mu, np.float64)[:, None]) / SIGMA
    nos = np.sum(np.abs(kk) < 1e-5, axis=1)
    kk = np.where((nos >= 0.5)[:, None], kk - 5e-5, kk)
    return np.sin(np.pi * kk) / (np.pi * kk)


def _composite_wav_weights(mu, proj_w, conv_w_i):
    """Fused 1->10ch 23-tap kernel E[o, s] (float64)."""
    krn = _sinc_rows(mu)                                  # [16,15]
    a = np.asarray(proj_w, np.float64)[:, 0, 0]           # [16]
    W = np.asarray(conv_w_i, np.float64)                  # [10,16,9]
    E = np.zeros((10, 23))
    for j in range(9):
        E[:, j:j + 15] += np.einsum('oc,cm->om', W[:, :, j] * a[None, :], krn)
    return E


def _eeg_lhsT(W1):
    """[128, 160]: cols g*80+(o*8+dt); row c*8+r; val W1[o,c,8g+r-dt]."""
    W1 = np.asarray(W1, np.float64)
    out = np.zeros((128, 160))
    g, c, r, o, dt = np.meshgrid(np.arange(2), np.arange(16), np.arange(8),
                                 np.arange(10), np.arange(8), indexing='ij')
    j = 8 * g + r - dt
    valid = (j >= 0) & (j < 9)
    out[(c * 8 + r)[valid], (g * 80 + o * 8 + dt)[valid]] = \
        W1[o[valid], c[valid], np.clip(j[valid], 0, 8)]
    return out.astype(np.float32)


def _wav_lhsT(E):
    """[36, 120]: row v*3+q, col o*12+dt, val E[o, 12q+v-dt]."""
    out = np.zeros((36, 120))
    v, q, o, dt = np.meshgrid(np.arange(12), np.arange(3), np.arange(10),
                              np.arange(12), indexing='ij')
    s = 12 * q + v - dt
    valid = (s >= 0) & (s < 23)
    out[(v * 3 + q)[valid], (o * 12 + dt)[valid]] = E[o[valid], np.clip(s[valid], 0, 22)]
    return out.astype(np.float32)


# --------------------------------------------------------------------------
# host-side per-core input slicing
# --------------------------------------------------------------------------

def _core_starts(k):
    return (min(k * 32767, NOUT - EEG_TC), min(k * 32767, NOUT - WAV_TC))


def _eeg_phases(eeg, k):
    """[128, 4097]: row c*8+r, col m = eeg[c, s_e + 8m + r]."""
    s_e, _ = _core_starts(k)
    v = eeg[:, s_e:s_e + 8 * EEG_COLS]                  # [16, 32776]
    p = v.reshape(16, EEG_COLS, 8).transpose(0, 2, 1)   # [16,8,4097]
    return p.reshape(128, EEG_COLS)


def _wav_phases(w_pad, k):
    """[36, 2731]: row v*3+q, col n = w_pad[s_w + 12(n+q) + v]."""
    _, s_w = _core_starts(k)
    sl = w_pad[s_w:s_w + 12 * (WAV_NCOL + 2)]
    y = sl.reshape(WAV_NCOL + 2, 12).T                  # y[v,m] = sl[12m+v]
    out = np.empty((36, WAV_NCOL), dtype=w_pad.dtype)
    for q in range(3):
        out[q::3, :] = y[:, q:q + WAV_NCOL]
    return out


# --------------------------------------------------------------------------
# bass kernel
# --------------------------------------------------------------------------

N_WARM_BIG = 10         # 512-col warmup matmuls (~0.43us each cold)
N_WARM_SMALL = 8        # 256-col trailing warmups (fine-grained handoff)


def _build_nc():
    import concourse.bacc as bacc
    import concourse.tile as tile
    import concourse.mybir as mybir

    f32 = mybir.dt.float32
    f16 = mybir.dt.float16
    nc = bacc.Bacc("TRN2", target_bir_lowering=False, debug=False,
                   num_devices=NCORES)

    # in0 = eeg lhsT [128,160] | eeg phase cols 0:2049, all fp8e4m3
    f8 = mybir.dt.float8e4
    in0 = nc.dram_tensor("in0", [128, 2209], f8, kind="ExternalInput")
    in1 = nc.dram_tensor("in1", [128, 2049], f8, kind="ExternalInput")
    # wavP = wav phases [36, 5462] | wavA lhsT [36,120] | wavB lhsT [36,120]
    wavP = nc.dram_tensor("wavP", [36, 2 * WAV_NCOL + 240], f8,
                          kind="ExternalInput")
    out = nc.dram_tensor("out", [128, 8], f16, kind="ExternalOutput")

    X = mybir.AxisListType.X
    Copy = mybir.ActivationFunctionType.Copy
    Max = mybir.AluOpType.max

    with tile.TileContext(nc) as tc:
        with (
            tc.tile_pool(name="sb", bufs=1) as sb,
            tc.tile_pool(name="ps", bufs=4, space="PSUM") as psp,
        ):
            # ---- input DMAs first; one dma_start per queue.
            in0_t = sb.tile([128, 2209], f8, tag="in0", name="in0t")
            in1_t = sb.tile([128, 2049], f8, tag="in1", name="in1t")
            wav_t = sb.tile([36, 2 * WAV_NCOL + 240], f8, tag="wav")
            scr = sb.tile([128, 512], f16, tag="scr")
            nc.gpsimd.memset(scr[:], 0.0)   # first: gates the PE warmups
            nc.sync.dma_start(in0_t[:], in0[:])
            nc.scalar.dma_start(in1_t[:], in1[:])
            nc.gpsimd.dma_start(wav_t[:], wavP[:])

            # ---- scratch / output tiles
            out16 = sb.tile([128, 8], f16, tag="out16")
            nc.gpsimd.memset(out16[:], 0.0)
            stg = [sb.tile([120, 1024], f16, tag=f"stg{i}", name=f"stg{i}")
                   for i in range(6)]          # ACT-cast staging
            tre = [sb.tile([120, 1024], f16, tag=f"tre{i}", name=f"tre{i}")
                   for i in range(3)]          # tree level-1 outputs
            rem = sb.tile([120, 3, 512], f16, tag="rem")  # tree remnants
            nc.gpsimd.memset(rem[:], 0.0)

            # ---- PE warmup: continuous activity to open the HAM clock gate.
            wps = psp.tile([120, 1024], f32, tag="ps", name="wps")
            for _ in range(N_WARM_BIG):
                nc.tensor.matmul(wps[0:80, 0:512], scr[:, 0:80], scr[:],
                                 start=True, stop=True)
            for _ in range(N_WARM_SMALL):
                nc.tensor.matmul(wps[0:80, 0:256], scr[:, 0:80], scr[:, 0:256],
                                 start=True, stop=True)

            wE = in0_t[:, 0:160]   # fp8 eeg lhsT

            def eeg_tile(src, base, name):
                """PSUM [80,1024] for eeg phase cols base:base+1024 of src."""
                ps = psp.tile([120, 1024], f32, tag="ps", name=name)
                for j in (0, 512):
                    for g in (0, 1):
                        nc.tensor.matmul(ps[0:80, j:j + 512],
                                         wE[:, 80 * g:80 * g + 80],
                                         src[:, base + j + g:base + j + g + 512],
                                         start=(g == 0), stop=(g == 1))
                return ps

            def wav_tile(si, c0, ncol, name):
                """PSUM [120,ncol] from stream si cols c0:c0+ncol."""
                ps = psp.tile([120, 1024], f32, tag="ps", name=name)
                lhs = wav_t[0:36, 2 * WAV_NCOL + 120 * si:
                            2 * WAV_NCOL + 120 * si + 120]
                j = 0
                while j < ncol:
                    nn = min(512, ncol - j)
                    nc.tensor.matmul(ps[:, j:j + nn], lhs,
                                     wav_t[:, si * WAV_NCOL + c0 + j:
                                           si * WAV_NCOL + c0 + j + nn],
                                     start=True, stop=True)
                    j += nn
                return ps

            # ---- production order = expected data-arrival order
            g0 = eeg_tile(in0_t, 160, "g0")
            g1 = eeg_tile(in0_t, 160 + 1024, "g1")
            a0 = wav_tile(0, 0, 1024, "a0")
            a1 = wav_tile(0, 1024, 1024, "a1")
            b0 = wav_tile(1, 0, 1024, "b0")
            b1 = wav_tile(1, 1024, 1024, "b1")
            at = wav_tile(0, 2048, 683, "at")
            bt = wav_tile(1, 2048, 683, "bt")
            g2 = eeg_tile(in1_t, 0, "g2")
            g3 = eeg_tile(in1_t, 1024, "g3")

            # ---- evacuation: ACT casts six tiles (rate-matches production
            # so PSUM WAR deps never stall the PE for long); DVE runs cheap
            # fp16 pair-max trees over the casts, direct-reduces the rest,
            # and one 3D-AP reduce folds all tree remnants.
            nc.scalar.activation(stg[0][0:80, :], g0[0:80, :], Copy)
            nc.scalar.activation(stg[1][0:80, :], g1[0:80, :], Copy)
            nc.scalar.activation(stg[2][:], a0[:], Copy)
            nc.scalar.activation(stg[3][:], a1[:], Copy)
            nc.scalar.activation(stg[4][:], b0[:], Copy)
            nc.scalar.activation(stg[5][:], b1[:], Copy)

            # out16 cols: 0=eeg tree, 1=wavA tree, 2=wavB tree, 3=at, 4=bt,
            #             5=g2, 6=g3
            TT = nc.vector.tensor_tensor
            TT(tre[0][0:80, :], stg[0][0:80, :], stg[1][0:80, :], op=Max)
            TT(rem[0:80, 0, :], tre[0][0:80, 0:512], tre[0][0:80, 512:1024],
               op=Max)
            TT(tre[1][:], stg[2][:], stg[3][:], op=Max)
            TT(rem[:, 1, :], tre[1][:, 0:512], tre[1][:, 512:1024], op=Max)
            nc.vector.reduce_max(out16[0:120, 3:4], at[:, 0:683], axis=X)
            TT(tre[2][:], stg[4][:], stg[5][:], op=Max)
            TT(rem[:, 2, :], tre[2][:, 0:512], tre[2][:, 512:1024], op=Max)
            nc.vector.reduce_max(out16[0:120, 4:5], bt[:, 0:683], axis=X)
            nc.vector.reduce_max(out16[0:120, 0:3], rem[:], axis=X)
            nc.vector.reduce_max(out16[0:80, 5:6], g2[0:80, :], axis=X)
            nc.vector.reduce_max(out16[0:80, 6:7], g3[0:80, :], axis=X)

            nc.sync.dma_start(out[:], out16[:])

    nc.compile()
    return nc


def _get_nc():
    if "nc" not in _NC_CACHE:
        _NC_CACHE["nc"] = _build_nc()
    return _NC_CACHE["nc"]


# --------------------------------------------------------------------------
# entry point
# --------------------------------------------------------------------------

def _prepare_in_maps(x, mu, projA_w, projB_w, conv_w):
    from ml_dtypes import float8_e4m3fn as f8np
    x = np.asarray(x, np.float32)
    eeg = np.ascontiguousarray(x[0, 0, 1:17, :]).astype(f8np)
    zt = np.zeros(64, np.float32)
    w_padA = np.concatenate([np.zeros(7, np.float32), x[0, 0, 0, :], zt]
                            ).astype(f8np)
    w_padB = np.concatenate([np.zeros(7, np.float32), x[0, 0, 17, :], zt]
                            ).astype(f8np)

    conv_w = np.asarray(conv_w)
    E_A = _composite_wav_weights(mu, projA_w, conv_w[0])
    E_B = _composite_wav_weights(mu, projB_w, conv_w[2])
    eeg_w8 = _eeg_lhsT(conv_w[1]).astype(f8np)              # [128,160] fp8
    wav_w = np.zeros((36, 240), np.float32)
    wav_w[:, 0:120] = _wav_lhsT(E_A)
    wav_w[:, 120:240] = _wav_lhsT(E_B)
    wav_w = wav_w.astype(f8np)

    in_maps = []
    for k in range(NCORES):
        ph = _eeg_phases(eeg, k)
        in0 = np.concatenate([eeg_w8, ph[:, 0:2049]], axis=1)
        wavp = np.concatenate([_wav_phases(w_padA, k), _wav_phases(w_padB, k),
                               wav_w], axis=1)
        in_maps.append({
            "in0": np.ascontiguousarray(in0),
            "in1": np.ascontiguousarray(ph[:, 2048:4097]),
            "wavP": np.ascontiguousarray(wavp),
        })
    return in_maps


def _head(percore, conv_b, fc1_w, fc1_b, fc2_w, fc2_b):
    m = percore.max(axis=0).astype(np.float64)
    eeg_o = m[0:80].reshape(10, 8).max(axis=1)
    wavA_o = m[80:200].reshape(10, 12).max(axis=1)
    wavB_o = m[200:320].reshape(10, 12).max(axis=1)
    conv_b = np.asarray(conv_b, np.float64)
    f = np.concatenate([np.maximum(wavA_o + conv_b[0], 0.0),
                        np.maximum(eeg_o + conv_b[1], 0.0),
                        np.maximum(wavB_o + conv_b[2], 0.0)])
    h = 1.0 / (1.0 + np.exp(-(f @ np.asarray(fc1_w, np.float64).T
                              + np.asarray(fc1_b, np.float64))))
    o = 1.0 / (1.0 + np.exp(-(h @ np.asarray(fc2_w, np.float64).T
                              + np.asarray(fc2_b, np.float64))))
    return o[None, :].astype(np.float32)


def _percore_from_out(arr):
    """Device 'out' [128,8] fp16 -> flat [320] (eeg 80, wavA 120, wavB 120).

    cols: 0=eeg tree, 1=wavA tree, 2=wavB tree, 3=at, 4=bt, 5=g2, 6=g3."""
    arr = np.asarray(arr, np.float32)
    return np.concatenate([arr[0:80, [0, 5, 6]].max(axis=1),
                           arr[0:120, [1, 3]].max(axis=1),
                           arr[0:120, [2, 4]].max(axis=1)])


def kernel(x, mu, projA_w, projB_w, conv_w, conv_b, fc1_w, fc1_b, fc2_w, fc2_b):
    global LAST_RESULT
    in_maps = _prepare_in_maps(x, mu, projA_w, projB_w, conv_w)
    nc = _get_nc()

    if os.environ.get("KERNEL_USE_SIM"):
        # sim mode for correctness checking without hardware
        from concourse.bass_interp import CoreSim
        percore = np.zeros((NCORES, 320), np.float32)
        for k in range(NCORES):
            sim = CoreSim(nc)
            for name, arr in in_maps[k].items():
                sim.tensor(name)[:] = arr
            sim.simulate()
            percore[k] = _percore_from_out(sim.tensor("out"))
    else:
        from concourse.bass_utils import run_bass_kernel_spmd
        trace = bool(os.environ.get("KERNEL_TRACE"))
        res = run_bass_kernel_spmd(nc, in_maps, list(range(NCORES)),
                                   trace=trace)
        LAST_RESULT = res
        percore = np.stack([_percore_from_out(res.results[k]["out"])
                            for k in range(NCORES)])

    return _head(percore, conv_b, fc1_w, fc1_b, fc2_w, fc2_b)
